# revision 17
# baseline (speedup 1.0000x reference)
"""LoRA multi-head attention on 8 TRN2 NeuronCores.

Sharding: data-parallel over batch (B=8 -> 1 batch element per core),
weights replicated, no collectives.

Host side (in kernel()): inputs are cast to bf16 and pre-transposed so
the device reads exactly the layouts the TensorEngine needs (the
contraction dim on partitions). LoRA B matrices are packed at 32-aligned
partition offsets (with duplicated rows where two matmuls must run
concurrently as PE row-tiles).

Device side per core, all bf16 with fp32 PSUM accumulation:
  qT = (WqT.T @ xT + BqT.T (AqT.T xT) / 16) / 8     [dout, n]
  kT likewise; v natural [n, dout] via (xT.T @ WvT), stored per-head
  with a ones column ([v_h | 1]) so PV also yields softmax denoms.
  Per head: S^T = kT_h.T qT_h -> exp (no max-sub; |s|=O(4)) -> PV;
  normalize via bf16 reciprocal-row broadcast matmuls (two concurrent
  PE tiles) instead of fp32 ones-outer-products.
  out = attnT.T @ WoT + lora + bo (bias via K=1 ones matmul).

PE warm-up runs ungated during the input DMA window (HAM reaches 8/8
before real work), paced by per-x-tile gated batches.
"""

import sys

if "/opt/trn_rl_repo" not in sys.path:
    sys.path.insert(0, "/opt/trn_rl_repo")

import numpy as np
import ml_dtypes

BF16 = ml_dtypes.bfloat16

N = 1024  # tokens
D = 1024  # model dim
H = 16    # heads
HD = 64   # head dim
R = 16    # lora rank
P = 128   # partitions
F = 512   # psum free-dim tile
NCORES = 8
SCALING = 1.0 / 16.0  # lora alpha/rank
SCALE = HD ** -0.5

_CACHE = {}


def _build():
    import concourse.bacc as bacc
    import concourse.mybir as mybir
    import concourse.tile as tile

    f32 = mybir.dt.float32
    bf16 = mybir.dt.bfloat16
    Exp = mybir.ActivationFunctionType.Exp

    nc = bacc.Bacc("TRN2", target_bir_lowering=False, debug=False)

    # all big params arrive pre-transposed, bf16, from the host
    xT_e = nc.declare_dram_parameter("xT", [D, N], bf16, isOutput=False)
    wT_e = {
        nm: nc.declare_dram_parameter(nm, [D, D], bf16, isOutput=False)
        for nm in ("WqT", "WkT", "WvT", "WoT")
    }
    a3_e = nc.declare_dram_parameter("A3T", [D, 96], bf16, isOutput=False)
    b3_e = nc.declare_dram_parameter("B3T", [112, D], bf16, isOutput=False)
    aT_e = {
        nm: nc.declare_dram_parameter(nm, [D, R], bf16, isOutput=False)
        for nm in ("AoT",)
    }
    bT_e = {
        nm: nc.declare_dram_parameter(nm, [49, D], bf16, isOutput=False)
        for nm in ("BoT",)
    }
    bm_e = nc.declare_dram_parameter("BM", [33, P], bf16, isOutput=False)
    out_e = nc.declare_dram_parameter("out", [N, D], bf16, isOutput=True)

    with tile.TileContext(nc) as tc:
        with (
            tc.tile_pool(name="wpool", bufs=1) as wpool,
            tc.tile_pool(name="stage", bufs=2) as stage,
            tc.tile_pool(name="ps", bufs=1, space="PSUM") as ps,
        ):
            qs = [nc.sync, nc.scalar, nc.gpsimd]
            xqs = qs

            # dummy for warm-up matmuls (must be initialized for the sim)
            wdummy = wpool.tile([P, P], bf16, tag="wdummy")
            nc.vector.memset(wdummy[:], 0.0)

            # ---- load pre-transposed tensors straight into SBUF ----
            T = {}
            aT = {}
            bT = {}
            qi = 0

            def load_big(nm, ext, queues=None):
                nonlocal qi
                queues = queues or qs
                T[nm] = []
                for t in range(8):
                    tt = wpool.tile([P, D], bf16, tag=f"T_{nm}_{t}",
                                    name=f"T_{nm}_{t}")
                    queues[qi % len(queues)].dma_start(
                        out=tt[:], in_=ext[t * P:(t + 1) * P, :])
                    qi += 1
                    T[nm].append(tt)

            def load_a(nm):
                nonlocal qi
                key = nm[:2]
                aT[key] = []
                for t in range(8):
                    tt = wpool.tile([P, R], bf16, tag=f"aT_{nm}_{t}",
                                    name=f"aT_{nm}_{t}")
                    qs[qi % 3].dma_start(out=tt[:],
                                         in_=aT_e[nm][t * P:(t + 1) * P, :])
                    qi += 1
                    aT[key].append(tt)

            def load_b(nm):
                nonlocal qi
                tt = wpool.tile([49, D], bf16, tag=f"bT_{nm}")
                qs[qi % 3].dma_start(out=tt[:], in_=bT_e[nm][:, :])
                qi += 1
                bT[nm[:2]] = tt

            a3 = []
            for t in range(8):
                tt = wpool.tile([P, 96], bf16, tag=f"a3_{t}",
                                name=f"a3_{t}")
                qs[qi % 3].dma_start(out=tt[:],
                                     in_=a3_e[t * P:(t + 1) * P, :])
                qi += 1
                a3.append(tt)
            b3 = wpool.tile([112, D], bf16, tag="b3")
            qs[qi % 3].dma_start(out=b3[:], in_=b3_e[:, :])
            qi += 1
            load_big("x", xT_e, queues=xqs)

            # ---- PE warm-up: ungated burst spans the DMA head, then
            # per-x-tile gated batches keep HAM at 8/8 until real work ----
            wps = ps.tile([P, F], f32, tag="pvpsum", bufs=2)
            for _ in range(120):
                nc.tensor.matmul(wps[:, 0:P], wdummy[:], wdummy[:],
                                 start=True, stop=True)
            for t in range(8):
                nbatch = (28, 32, 32, 32, 28, 24, 20, 16)[t]
                for _ in range(nbatch):
                    nc.tensor.matmul(wps[:, 0:P], wdummy[:],
                                     T["x"][t][:, 0:P], start=True, stop=True)

            load_big("Wv", wT_e["WvT"])
            load_big("Wq", wT_e["WqT"])
            load_big("Wk", wT_e["WkT"])
            load_big("Wo", wT_e["WoT"])
            load_a("AoT")
            load_b("BoT")
            onesb = wpool.tile([P, HD], bf16, tag="onesb")
            nc.vector.memset(onesb[:], 1.0)
            for _ in range(2):
                dzz = stage.tile([33, F], f32, tag="dsrc", bufs=2)
                nc.vector.memset(dzz[:], 1.0)
            # block mask [2,128]: row 0 selects out partitions 0-63,
            # row 1 selects 64-127 -- one K=2 matmul broadcasts two
            # reciprocal rows to the two head-halves of a psum tile


            # ---- lora intermediates, q/k/v packed at 32-aligned rows
            # (host ships A3T/B3T with Aq@0, Ak@32, Av@64, Av-dup@96) ----
            tsb3 = []
            for nh in range(2):
                ns = slice(nh * F, (nh + 1) * F)
                pt = ps.tile([96, F], f32, tag="tpsum", bufs=1)
                for kt in range(8):
                    nc.tensor.matmul(pt[:], a3[kt][:], T["x"][kt][:, ns],
                                     start=(kt == 0), stop=(kt == 7))
                t_s = stage.tile([112, F], bf16, tag="tsb", bufs=2,
                                 name=f"tsb3_{nh}")
                nc.vector.tensor_scalar_mul(t_s[0:96, :], pt[:], SCALING)
                nc.vector.tensor_scalar_mul(t_s[96:112, :], pt[64:80, :],
                                            SCALING)
                tsb3.append(t_s)

            # ---- v natural, per-head layout [v_h | 1], with the dt=0
            # projection woven in so attention starts immediately after ----
            qks = {}

            def proj_gen(dt):
                qk = {}
                for nm, wnm, bnm, scl in (("q", "Wq", "Bq", SCALE),
                                          ("k", "Wk", "Bk", None)):
                    dst = wpool.tile([P, D], bf16, tag=f"{nm}T",
                                     bufs=3, name=f"{nm}T_{dt}")
                    qk[nm] = dst
                    for nh in range(2):
                        ns = slice(nh * F, (nh + 1) * F)
                        pq = ps.tile([P, F], f32, tag="projpsum", bufs=1)
                        for kt in range(8):
                            nc.tensor.matmul(
                                pq[:], T[wnm][kt][:, dt * P:(dt + 1) * P],
                                T["x"][kt][:, ns],
                                start=(kt == 0), stop=False)
                            yield
                        ro3 = 0 if nm == "q" else 32
                        nc.tensor.matmul(pq[:],
                                         b3[ro3:ro3 + R,
                                            dt * P:(dt + 1) * P],
                                         tsb3[nh][ro3:ro3 + R, :],
                                         start=False, stop=True)
                        yield
                        if scl is None:
                            nc.vector.tensor_copy(dst[:, ns], pq[:])
                        else:
                            nc.vector.tensor_scalar_mul(dst[:, ns],
                                                        pq[:], scl)
                        yield
                qks[dt] = qk

            VW = H * (HD + 1)  # 1040
            v_sb = [wpool.tile([P, VW], bf16, tag=f"v_{t}",
                               name=f"v_{t}") for t in range(8)]
            g0 = proj_gen(0)
            for nt in range(8):
                vr = v_sb[nt][:].rearrange("p (h c) -> p h c", c=HD + 1)
                pvs = []
                for dh in range(2):
                    ds = slice(dh * F, (dh + 1) * F)
                    pv = ps.tile([P, F], f32,
                                 tag=("spair" if dh == 0 else "pvpsum"),
                                 bufs=2)
                    pvs.append(pv)
                    for kt in range(8):
                        nc.tensor.matmul(
                            pv[:], T["x"][kt][:, nt * P:(nt + 1) * P],
                            T["Wv"][kt][:, ds], start=(kt == 0), stop=False)
                    for _ in range(3):
                        next(g0, None)
                # v-lora pair: tiles (64,0) and (96,0), concurrent
                nc.tensor.matmul(
                    pvs[0][:],
                    tsb3[nt // 4][64:80, (nt % 4) * P:(nt % 4 + 1) * P],
                    b3[64:80, 0:F], start=False, stop=True)
                nc.tensor.matmul(
                    pvs[1][:],
                    tsb3[nt // 4][96:112, (nt % 4) * P:(nt % 4 + 1) * P],
                    b3[96:112, F:2 * F], start=False, stop=True,
                    tile_position=(96, 0))
                for dh in range(2):
                    pvr = pvs[dh][:].rearrange("p (h c) -> p h c", c=HD)
                    nc.vector.tensor_copy(vr[:, dh * 8:(dh + 1) * 8, 0:HD],
                                          pvr[:])
                    for _ in range(2):
                        next(g0, None)
                nc.vector.memset(vr[:, :, HD:HD + 1], 1.0)
            for _ in g0:
                pass

            # ---- per dout-tile: qT, kT, then its 2 heads' attention.
            # The NEXT tile's projection matmuls are woven into the
            # attention inner loop (generator) so the PE stays dense
            # while ACT runs the exps -- keeps HAM at K=8/8. ----
            attnT = [wpool.tile([P, D], bf16, tag=f"attnT_{t}",
                                name=f"attnT_{t}") for t in range(8)]
            for dt in range(8):
                g = proj_gen(dt + 1) if dt < 7 else iter(())
                h0 = 2 * dt
                qt = qks[dt]["q"]
                ktt = qks[dt]["k"]
                for nh in range(2):
                    ns = slice(nh * F, (nh + 1) * F)
                    po = {}
                    for h in (h0, h0 + 1):
                        po[h] = ps.tile([HD + 1, F], f32, tag="pvpsum",
                                        bufs=2, name=f"po_{h}_{nh}")
                    for mt in range(8):
                        spair = ps.tile([P, 2 * F], f32, tag="spair",
                                        bufs=2)
                        for hi, h in enumerate((h0, h0 + 1)):
                            ro = (h % 2) * HD
                            m0 = mt * P
                            nc.tensor.matmul(
                                spair[:, hi * F:(hi + 1) * F],
                                ktt[ro:ro + HD, m0:m0 + P],
                                qt[ro:ro + HD, ns], start=True, stop=True)
                        pte = stage.tile([P, 2 * F], bf16, tag="pt", bufs=3)
                        nc.scalar.activation(pte[:], spair[:], Exp)
                        for hi, h in enumerate((h0, h0 + 1)):
                            nc.tensor.matmul(
                                po[h][:],
                                v_sb[mt][:, h * (HD + 1):(h + 1) * (HD + 1)],
                                pte[:, hi * F:(hi + 1) * F],
                                start=(mt == 0), stop=(mt == 7))
                        for _ in range(3):
                            next(g, None)
                    # normalize: copy po -> SBUF immediately (frees the
                    # pvpsum slot), batched recip+cast of both denom rows,
                    # K=1 bf16 broadcasts, multiply oah(SBUF) x pb(PSUM)
                    oahs = {}
                    dsrc = stage.tile([33, F], f32, tag="dsrc", bufs=2)
                    drow = stage.tile([33, F], f32, tag="drow", bufs=2)
                    drb = stage.tile([33, F], bf16, tag="drb", bufs=2)
                    for hi, h in enumerate((h0, h0 + 1)):
                        oah = stage.tile([HD + 1, F], f32, tag="oah",
                                         bufs=3)
                        nc.vector.tensor_copy(oah[:], po[h][:])
                        oahs[h] = oah
                        nc.vector.tensor_copy(dsrc[32 * hi:32 * hi + 1, :],
                                              oah[HD:HD + 1, :])
                        next(g, None)
                    nc.vector.reciprocal_approx_fast(drow[:], dsrc[:])
                    nc.vector.tensor_copy(drb[:], drow[:])
                    next(g, None)
                    for hi, h in enumerate((h0, h0 + 1)):
                        ro = (h % 2) * HD
                        r0 = 32 * hi
                        pb = ps.tile([HD, F], f32, tag="tpsum", bufs=1)
                        nc.tensor.matmul(pb[:], onesb[r0:r0 + 1, 0:HD],
                                         drb[r0:r0 + 1, :],
                                         start=True, stop=True)
                        ast = stage.tile([HD, F], bf16, tag="ast", bufs=3)
                        nc.vector.tensor_mul(ast[:], oahs[h][0:HD, :],
                                             pb[:])
                        nc.sync.dma_start(out=attnT[dt][ro:ro + HD, ns],
                                          in_=ast[:])
                        for _ in range(2):
                            next(g, None)
                for _ in g:
                    pass

            # ---- output projection ----
            # to rows: 0-15 Ao-lora (nh cols), 16 ones (bias row),
            # 32-48 duplicate so Bo-lora pairs run as concurrent tiles
            to = wpool.tile([49, D], bf16, tag="toT")
            nc.vector.memset(to[:], 1.0)
            pt_o = ps.tile([48, F], f32, tag="tpsum", bufs=1)
            for kt in range(8):
                nc.tensor.matmul(pt_o[0:R, :], aT["Ao"][kt][:],
                                 attnT[kt][:, 0:F],
                                 start=(kt == 0), stop=(kt == 7))
                nc.tensor.matmul(pt_o[32:32 + R, :], aT["Ao"][kt][:],
                                 attnT[kt][:, F:2 * F],
                                 start=(kt == 0), stop=(kt == 7))
            nc.vector.tensor_scalar_mul(to[0:R, 0:F], pt_o[0:R, :], SCALING)
            nc.vector.tensor_scalar_mul(to[0:R, F:2 * F],
                                        pt_o[32:32 + R, :], SCALING)
            nc.vector.tensor_copy(to[32:32 + R, 0:F], to[0:R, 0:F])
            nc.vector.tensor_copy(to[32:32 + R, F:2 * F], to[0:R, F:2 * F])
            for nt in range(8):
                nslc = slice(nt * P, (nt + 1) * P)
                pfs = []
                for dh in range(2):
                    ds = slice(dh * F, (dh + 1) * F)
                    pf = ps.tile([P, F], f32,
                                 tag=("spair" if dh == 0 else "pvpsum"),
                                 bufs=2)
                    pfs.append(pf)
                    for kt in range(8):
                        nc.tensor.matmul(pf[:], attnT[kt][:, nslc],
                                         T["Wo"][kt][:, ds],
                                         start=(kt == 0), stop=False)
                # Bo-lora pair: tiles (0,0) and (32,0), concurrent
                nc.tensor.matmul(pfs[0][:], to[0:R + 1, nslc],
                                 bT["Bo"][0:R + 1, 0:F],
                                 start=False, stop=True)
                nc.tensor.matmul(pfs[1][:], to[32:33 + R, nslc],
                                 bT["Bo"][32:33 + R, F:2 * F],
                                 start=False, stop=True)
                for dh in range(2):
                    ds = slice(dh * F, (dh + 1) * F)
                    osb = stage.tile([P, F], bf16, tag="osb")
                    nc.vector.tensor_copy(osb[:], pfs[dh][:])
                    nc.sync.dma_start(out=out_e[nslc, ds], in_=osb[:])
    nc.compile()
    return nc


def _get_nc():
    if "nc" not in _CACHE:
        _CACHE["nc"] = _build()
    return _CACHE["nc"]


def _prep_shared(inputs):
    def tb(a):  # transpose + bf16, contiguous
        return np.ascontiguousarray(np.asarray(a, np.float32).T.astype(BF16))

    shared = {}
    for nm in ("Wq", "Wk", "Wv", "Wo", "Ao"):
        shared[nm + "T"] = tb(inputs[nm])
    boa = np.zeros((49, D), np.float32)
    boa[0:R] = np.asarray(inputs["Bo"], np.float32).T
    boa[R] = np.asarray(inputs["bo"], np.float32)
    boa[32:32 + R] = boa[0:R]
    boa[32 + R] = boa[R]
    shared["BoT"] = np.ascontiguousarray(boa.astype(BF16))
    a3 = np.zeros((D, 96), np.float32)
    b3 = np.zeros((112, D), np.float32)
    for j, nm in enumerate(("q", "k", "v")):
        a3[:, 32 * j:32 * j + R] = np.asarray(inputs["A" + nm], np.float32).T
        b3[32 * j:32 * j + R, :] = np.asarray(inputs["B" + nm], np.float32).T
    b3[96:96 + R] = b3[64:64 + R]
    shared["A3T"] = np.ascontiguousarray(a3.astype(BF16))
    bm = np.zeros((33, P), np.float32)
    bm[0, 0:HD] = 1.0
    bm[32, HD:P] = 1.0
    shared["BM"] = np.ascontiguousarray(bm.astype(BF16))
    shared["B3T"] = np.ascontiguousarray(b3.astype(BF16))
    return shared


def kernel(**inputs):
    from concourse import bass_utils

    nc = _get_nc()
    shared = _prep_shared(inputs)
    x = np.asarray(inputs["x"], np.float32)
    in_maps = []
    for i in range(NCORES):
        m = dict(shared)
        m["xT"] = np.ascontiguousarray(x[i].T.astype(BF16))
        in_maps.append(m)
    res = bass_utils.run_bass_kernel_spmd(nc, in_maps,
                                          core_ids=list(range(NCORES)))
    return np.stack([np.asarray(res.results[i]["out"]).astype(np.float32)
                     for i in range(NCORES)], axis=0)


# revision 18
# speedup vs baseline: 1.2186x; 1.2186x over previous
"""LoRA multi-head attention on 8 TRN2 NeuronCores.

Sharding: data-parallel over batch (B=8 -> 1 batch element per core),
weights replicated, no collectives.

Host side (in kernel()): inputs are cast to bf16 and pre-transposed so
the device reads exactly the layouts the TensorEngine needs (the
contraction dim on partitions). LoRA B matrices are packed at 32-aligned
partition offsets (with duplicated rows where two matmuls must run
concurrently as PE row-tiles).

Device side per core, all bf16 with fp32 PSUM accumulation:
  qT = (WqT.T @ xT + BqT.T (AqT.T xT) / 16) / 8     [dout, n]
  kT likewise; v natural [n, dout] via (xT.T @ WvT), stored per-head
  with a ones column ([v_h | 1]) so PV also yields softmax denoms.
  Per head: S^T = kT_h.T qT_h -> exp (no max-sub; |s|=O(4)) -> PV;
  normalize via bf16 reciprocal-row broadcast matmuls (two concurrent
  PE tiles) instead of fp32 ones-outer-products.
  out = attnT.T @ WoT + lora + bo (bias via K=1 ones matmul).

PE warm-up runs ungated during the input DMA window (HAM reaches 8/8
before real work), paced by per-x-tile gated batches.
"""

import sys

if "/opt/trn_rl_repo" not in sys.path:
    sys.path.insert(0, "/opt/trn_rl_repo")

import numpy as np
import ml_dtypes

BF16 = ml_dtypes.bfloat16

N = 1024  # tokens
D = 1024  # model dim
H = 16    # heads
HD = 64   # head dim
R = 16    # lora rank
P = 128   # partitions
F = 512   # psum free-dim tile
NCORES = 8
SCALING = 1.0 / 16.0  # lora alpha/rank
SCALE = HD ** -0.5

_CACHE = {}


def _build():
    import concourse.bacc as bacc
    import concourse.mybir as mybir
    import concourse.tile as tile

    f32 = mybir.dt.float32
    bf16 = mybir.dt.bfloat16
    Exp = mybir.ActivationFunctionType.Exp

    nc = bacc.Bacc("TRN2", target_bir_lowering=False, debug=False)

    # all big params arrive pre-transposed, bf16, from the host
    xT_e = nc.declare_dram_parameter("xT", [D, N], bf16, isOutput=False)
    wT_e = {
        nm: nc.declare_dram_parameter(nm, [D, D], bf16, isOutput=False)
        for nm in ("WqT", "WkT", "WvT", "WoT")
    }
    a3_e = nc.declare_dram_parameter("A3T", [D, 96], bf16, isOutput=False)
    b3_e = nc.declare_dram_parameter("B3T", [112, D], bf16, isOutput=False)
    aT_e = {
        nm: nc.declare_dram_parameter(nm, [D, R], bf16, isOutput=False)
        for nm in ("AoT",)
    }
    bT_e = {
        nm: nc.declare_dram_parameter(nm, [49, D], bf16, isOutput=False)
        for nm in ("BoT",)
    }
    bm_e = nc.declare_dram_parameter("BM", [33, P], bf16, isOutput=False)
    out_e = nc.declare_dram_parameter("out", [N, D], bf16, isOutput=True)

    with tile.TileContext(nc) as tc:
        with (
            tc.tile_pool(name="wpool", bufs=1) as wpool,
            tc.tile_pool(name="stage", bufs=2) as stage,
            tc.tile_pool(name="ps", bufs=1, space="PSUM") as ps,
        ):
            qs = [nc.sync, nc.scalar, nc.gpsimd]
            xqs = qs

            # dummy for warm-up matmuls (must be initialized for the sim)
            wdummy = wpool.tile([P, P], bf16, tag="wdummy")
            nc.vector.memset(wdummy[:], 0.0)

            # ---- load pre-transposed tensors straight into SBUF ----
            T = {}
            aT = {}
            bT = {}
            qi = 0

            def load_big(nm, ext, queues=None):
                nonlocal qi
                queues = queues or qs
                T[nm] = []
                for t in range(8):
                    tt = wpool.tile([P, D], bf16, tag=f"T_{nm}_{t}",
                                    name=f"T_{nm}_{t}")
                    queues[qi % len(queues)].dma_start(
                        out=tt[:], in_=ext[t * P:(t + 1) * P, :])
                    qi += 1
                    T[nm].append(tt)

            def load_a(nm):
                nonlocal qi
                key = nm[:2]
                aT[key] = []
                for t in range(8):
                    tt = wpool.tile([P, R], bf16, tag=f"aT_{nm}_{t}",
                                    name=f"aT_{nm}_{t}")
                    qs[qi % 3].dma_start(out=tt[:],
                                         in_=aT_e[nm][t * P:(t + 1) * P, :])
                    qi += 1
                    aT[key].append(tt)

            def load_b(nm):
                nonlocal qi
                tt = wpool.tile([49, D], bf16, tag=f"bT_{nm}")
                qs[qi % 3].dma_start(out=tt[:], in_=bT_e[nm][:, :])
                qi += 1
                bT[nm[:2]] = tt

            a3 = []
            for t in range(8):
                tt = wpool.tile([P, 96], bf16, tag=f"a3_{t}",
                                name=f"a3_{t}")
                qs[qi % 3].dma_start(out=tt[:],
                                     in_=a3_e[t * P:(t + 1) * P, :])
                qi += 1
                a3.append(tt)
            b3 = wpool.tile([112, D], bf16, tag="b3")
            qs[qi % 3].dma_start(out=b3[:], in_=b3_e[:, :])
            qi += 1
            load_big("x", xT_e, queues=xqs)

            # ---- PE warm-up: ungated burst spans the DMA head, then
            # per-x-tile gated batches keep HAM at 8/8 until real work ----
            wps = ps.tile([P, F], f32, tag="pvpsum", bufs=2)
            for _ in range(120):
                nc.tensor.matmul(wps[:, 0:P], wdummy[:], wdummy[:],
                                 start=True, stop=True)
            for t in range(8):
                nbatch = (28, 32, 32, 32, 28, 24, 20, 16)[t]
                for _ in range(nbatch):
                    nc.tensor.matmul(wps[:, 0:P], wdummy[:],
                                     T["x"][t][:, 0:P], start=True, stop=True)

            load_big("Wv", wT_e["WvT"])
            load_big("Wq", wT_e["WqT"])
            load_big("Wk", wT_e["WkT"])
            load_big("Wo", wT_e["WoT"])
            load_a("AoT")
            load_b("BoT")
            onesb = wpool.tile([P, HD], bf16, tag="onesb")
            nc.vector.memset(onesb[:], 1.0)
            # block mask [2,128]: row 0 selects out partitions 0-63,
            # row 1 selects 64-127 -- one K=2 matmul broadcasts two
            # reciprocal rows to the two head-halves of a psum tile


            # ---- lora intermediates, q/k/v packed at 32-aligned rows
            # (host ships A3T/B3T with Aq@0, Ak@32, Av@64, Av-dup@96) ----
            tsb3 = []
            for nh in range(2):
                ns = slice(nh * F, (nh + 1) * F)
                pt = ps.tile([96, F], f32, tag="tpsum", bufs=1)
                for kt in range(8):
                    nc.tensor.matmul(pt[:], a3[kt][:], T["x"][kt][:, ns],
                                     start=(kt == 0), stop=(kt == 7))
                t_s = stage.tile([112, F], bf16, tag="tsb", bufs=2,
                                 name=f"tsb3_{nh}")
                nc.vector.tensor_scalar_mul(t_s[0:96, :], pt[:], SCALING)
                nc.vector.tensor_scalar_mul(t_s[96:112, :], pt[64:80, :],
                                            SCALING)
                tsb3.append(t_s)

            # ---- v natural, per-head layout [v_h | 1], with the dt=0
            # projection woven in so attention starts immediately after ----
            qks = {}

            def proj_gen(dt):
                qk = {}
                for nm, wnm, bnm, scl in (("q", "Wq", "Bq", SCALE),
                                          ("k", "Wk", "Bk", None)):
                    dst = wpool.tile([P, D], bf16, tag=f"{nm}T",
                                     bufs=3, name=f"{nm}T_{dt}")
                    qk[nm] = dst
                    for nh in range(2):
                        ns = slice(nh * F, (nh + 1) * F)
                        pq = ps.tile([P, F], f32, tag="projpsum", bufs=1)
                        for kt in range(8):
                            nc.tensor.matmul(
                                pq[:], T[wnm][kt][:, dt * P:(dt + 1) * P],
                                T["x"][kt][:, ns],
                                start=(kt == 0), stop=False)
                            yield
                        ro3 = 0 if nm == "q" else 32
                        nc.tensor.matmul(pq[:],
                                         b3[ro3:ro3 + R,
                                            dt * P:(dt + 1) * P],
                                         tsb3[nh][ro3:ro3 + R, :],
                                         start=False, stop=True)
                        yield
                        if scl is None:
                            nc.vector.tensor_copy(dst[:, ns], pq[:])
                        else:
                            nc.vector.tensor_scalar_mul(dst[:, ns],
                                                        pq[:], scl)
                        yield
                qks[dt] = qk

            VW = H * (HD + 1)  # 1040
            v_sb = [wpool.tile([P, VW], bf16, tag=f"v_{t}",
                               name=f"v_{t}") for t in range(8)]
            g0 = proj_gen(0)
            for nt in range(8):
                vr = v_sb[nt][:].rearrange("p (h c) -> p h c", c=HD + 1)
                pvs = []
                for dh in range(2):
                    ds = slice(dh * F, (dh + 1) * F)
                    pv = ps.tile([P, F], f32,
                                 tag=("spair" if dh == 0 else "pvpsum"),
                                 bufs=2)
                    pvs.append(pv)
                    for kt in range(8):
                        nc.tensor.matmul(
                            pv[:], T["x"][kt][:, nt * P:(nt + 1) * P],
                            T["Wv"][kt][:, ds], start=(kt == 0), stop=False)
                    for _ in range(3):
                        next(g0, None)
                # v-lora pair: tiles (64,0) and (96,0), concurrent
                nc.tensor.matmul(
                    pvs[0][:],
                    tsb3[nt // 4][64:80, (nt % 4) * P:(nt % 4 + 1) * P],
                    b3[64:80, 0:F], start=False, stop=True)
                nc.tensor.matmul(
                    pvs[1][:],
                    tsb3[nt // 4][96:112, (nt % 4) * P:(nt % 4 + 1) * P],
                    b3[96:112, F:2 * F], start=False, stop=True,
                    tile_position=(96, 0))
                for dh in range(2):
                    pvr = pvs[dh][:].rearrange("p (h c) -> p h c", c=HD)
                    nc.vector.tensor_copy(vr[:, dh * 8:(dh + 1) * 8, 0:HD],
                                          pvr[:])
                    for _ in range(2):
                        next(g0, None)
                nc.vector.memset(vr[:, :, HD:HD + 1], 1.0)
            for _ in g0:
                pass

            # ---- per dout-tile: qT, kT, then its 2 heads' attention.
            # The NEXT tile's projection matmuls are woven into the
            # attention inner loop (generator) so the PE stays dense
            # while ACT runs the exps -- keeps HAM at K=8/8. ----
            attnT = [wpool.tile([P, D], bf16, tag=f"attnT_{t}",
                                name=f"attnT_{t}") for t in range(8)]
            for dt in range(8):
                g = proj_gen(dt + 1) if dt < 7 else iter(())
                h0 = 2 * dt
                qt = qks[dt]["q"]
                ktt = qks[dt]["k"]
                for nh in range(2):
                    ns = slice(nh * F, (nh + 1) * F)
                    po = {}
                    for h in (h0, h0 + 1):
                        po[h] = ps.tile([HD + 1, F], f32, tag="pvpsum",
                                        bufs=2, name=f"po_{h}_{nh}")
                    for mt in range(8):
                        spair = ps.tile([P, 2 * F], f32, tag="spair",
                                        bufs=2)
                        for hi, h in enumerate((h0, h0 + 1)):
                            ro = (h % 2) * HD
                            m0 = mt * P
                            nc.tensor.matmul(
                                spair[:, hi * F:(hi + 1) * F],
                                ktt[ro:ro + HD, m0:m0 + P],
                                qt[ro:ro + HD, ns], start=True, stop=True)
                        pte = stage.tile([P, 2 * F], bf16, tag="pt", bufs=3)
                        nc.scalar.activation(pte[:], spair[:], Exp)
                        for hi, h in enumerate((h0, h0 + 1)):
                            nc.tensor.matmul(
                                po[h][:],
                                v_sb[mt][:, h * (HD + 1):(h + 1) * (HD + 1)],
                                pte[:, hi * F:(hi + 1) * F],
                                start=(mt == 0), stop=(mt == 7))
                        for _ in range(3):
                            next(g, None)
                    # normalize (baseline chain, bf16 broadcast):
                    # oah copy frees po; bf16 denom row -> K=1 broadcast
                    # (1 HW matmul vs 2 for fp32); recip fp32; multiply
                    for hi, h in enumerate((h0, h0 + 1)):
                        ro = (h % 2) * HD
                        oah = stage.tile([HD + 1, F], f32, tag="oah",
                                         bufs=3)
                        nc.vector.tensor_copy(oah[:], po[h][:])
                        dnb = stage.tile([1, F], bf16, tag="dnb", bufs=3)
                        nc.vector.tensor_copy(dnb[:], oah[HD:HD + 1, :])
                        pb = ps.tile([HD, F], f32, tag="tpsum", bufs=1)
                        nc.tensor.matmul(pb[:], onesb[0:1, 0:HD], dnb[:],
                                         start=True, stop=True)
                        pbs = stage.tile([HD, F], f32, tag="pbs", bufs=3)
                        nc.vector.reciprocal_approx_fast(pbs[:], pb[:])
                        ast = stage.tile([HD, F], bf16, tag="ast", bufs=3)
                        nc.vector.tensor_mul(ast[:], oah[0:HD, :], pbs[:])
                        nc.sync.dma_start(out=attnT[dt][ro:ro + HD, ns],
                                          in_=ast[:])
                        for _ in range(2):
                            next(g, None)
                for _ in g:
                    pass

            # ---- output projection ----
            # to rows: 0-15 Ao-lora (nh cols), 16 ones (bias row),
            # 32-48 duplicate so Bo-lora pairs run as concurrent tiles
            to = wpool.tile([49, D], bf16, tag="toT")
            nc.vector.memset(to[:], 1.0)
            pt_o = ps.tile([48, F], f32, tag="tpsum", bufs=1)
            for kt in range(8):
                nc.tensor.matmul(pt_o[0:R, :], aT["Ao"][kt][:],
                                 attnT[kt][:, 0:F],
                                 start=(kt == 0), stop=(kt == 7))
                nc.tensor.matmul(pt_o[32:32 + R, :], aT["Ao"][kt][:],
                                 attnT[kt][:, F:2 * F],
                                 start=(kt == 0), stop=(kt == 7))
            nc.vector.tensor_scalar_mul(to[0:R, 0:F], pt_o[0:R, :], SCALING)
            nc.vector.tensor_scalar_mul(to[0:R, F:2 * F],
                                        pt_o[32:32 + R, :], SCALING)
            nc.vector.tensor_copy(to[32:32 + R, 0:F], to[0:R, 0:F])
            nc.vector.tensor_copy(to[32:32 + R, F:2 * F], to[0:R, F:2 * F])
            for nt in range(8):
                nslc = slice(nt * P, (nt + 1) * P)
                pfs = []
                for dh in range(2):
                    ds = slice(dh * F, (dh + 1) * F)
                    pf = ps.tile([P, F], f32,
                                 tag=("spair" if dh == 0 else "pvpsum"),
                                 bufs=2)
                    pfs.append(pf)
                    for kt in range(8):
                        nc.tensor.matmul(pf[:], attnT[kt][:, nslc],
                                         T["Wo"][kt][:, ds],
                                         start=(kt == 0), stop=False)
                # Bo-lora pair: tiles (0,0) and (32,0), concurrent
                nc.tensor.matmul(pfs[0][:], to[0:R + 1, nslc],
                                 bT["Bo"][0:R + 1, 0:F],
                                 start=False, stop=True)
                nc.tensor.matmul(pfs[1][:], to[32:33 + R, nslc],
                                 bT["Bo"][32:33 + R, F:2 * F],
                                 start=False, stop=True)
                for dh in range(2):
                    ds = slice(dh * F, (dh + 1) * F)
                    osb = stage.tile([P, F], bf16, tag="osb")
                    nc.vector.tensor_copy(osb[:], pfs[dh][:])
                    nc.sync.dma_start(out=out_e[nslc, ds], in_=osb[:])
    nc.compile()
    return nc


def _get_nc():
    if "nc" not in _CACHE:
        _CACHE["nc"] = _build()
    return _CACHE["nc"]


def _prep_shared(inputs):
    def tb(a):  # transpose + bf16, contiguous
        return np.ascontiguousarray(np.asarray(a, np.float32).T.astype(BF16))

    shared = {}
    for nm in ("Wq", "Wk", "Wv", "Wo", "Ao"):
        shared[nm + "T"] = tb(inputs[nm])
    boa = np.zeros((49, D), np.float32)
    boa[0:R] = np.asarray(inputs["Bo"], np.float32).T
    boa[R] = np.asarray(inputs["bo"], np.float32)
    boa[32:32 + R] = boa[0:R]
    boa[32 + R] = boa[R]
    shared["BoT"] = np.ascontiguousarray(boa.astype(BF16))
    a3 = np.zeros((D, 96), np.float32)
    b3 = np.zeros((112, D), np.float32)
    for j, nm in enumerate(("q", "k", "v")):
        a3[:, 32 * j:32 * j + R] = np.asarray(inputs["A" + nm], np.float32).T
        b3[32 * j:32 * j + R, :] = np.asarray(inputs["B" + nm], np.float32).T
    b3[96:96 + R] = b3[64:64 + R]
    shared["A3T"] = np.ascontiguousarray(a3.astype(BF16))
    bm = np.zeros((33, P), np.float32)
    bm[0, 0:HD] = 1.0
    bm[32, HD:P] = 1.0
    shared["BM"] = np.ascontiguousarray(bm.astype(BF16))
    shared["B3T"] = np.ascontiguousarray(b3.astype(BF16))
    return shared


def kernel(**inputs):
    from concourse import bass_utils

    nc = _get_nc()
    shared = _prep_shared(inputs)
    x = np.asarray(inputs["x"], np.float32)
    in_maps = []
    for i in range(NCORES):
        m = dict(shared)
        m["xT"] = np.ascontiguousarray(x[i].T.astype(BF16))
        in_maps.append(m)
    res = bass_utils.run_bass_kernel_spmd(nc, in_maps,
                                          core_ids=list(range(NCORES)))
    return np.stack([np.asarray(res.results[i]["out"]).astype(np.float32)
                     for i in range(NCORES)], axis=0)


# revision 19
# speedup vs baseline: 1.2350x; 1.0135x over previous
"""LoRA multi-head attention on 8 TRN2 NeuronCores.

Sharding: data-parallel over batch (B=8 -> 1 batch element per core),
weights replicated, no collectives.

Host side (in kernel()): inputs are cast to bf16 and pre-transposed so
the device reads exactly the layouts the TensorEngine needs (the
contraction dim on partitions). LoRA B matrices are packed at 32-aligned
partition offsets (with duplicated rows where two matmuls must run
concurrently as PE row-tiles).

Device side per core, all bf16 with fp32 PSUM accumulation:
  qT = (WqT.T @ xT + BqT.T (AqT.T xT) / 16) / 8     [dout, n]
  kT likewise; v natural [n, dout] via (xT.T @ WvT), stored per-head
  with a ones column ([v_h | 1]) so PV also yields softmax denoms.
  Per head: S^T = kT_h.T qT_h -> exp (no max-sub; |s|=O(4)) -> PV;
  normalize via bf16 reciprocal-row broadcast matmuls (two concurrent
  PE tiles) instead of fp32 ones-outer-products.
  out = attnT.T @ WoT + lora + bo (bias via K=1 ones matmul).

PE warm-up runs ungated during the input DMA window (HAM reaches 8/8
before real work), paced by per-x-tile gated batches.
"""

import sys

if "/opt/trn_rl_repo" not in sys.path:
    sys.path.insert(0, "/opt/trn_rl_repo")

import numpy as np
import ml_dtypes

BF16 = ml_dtypes.bfloat16

N = 1024  # tokens
D = 1024  # model dim
H = 16    # heads
HD = 64   # head dim
R = 16    # lora rank
P = 128   # partitions
F = 512   # psum free-dim tile
NCORES = 8
SCALING = 1.0 / 16.0  # lora alpha/rank
SCALE = HD ** -0.5

_CACHE = {}


def _build():
    import concourse.bacc as bacc
    import concourse.mybir as mybir
    import concourse.tile as tile

    f32 = mybir.dt.float32
    bf16 = mybir.dt.bfloat16
    Exp = mybir.ActivationFunctionType.Exp

    nc = bacc.Bacc("TRN2", target_bir_lowering=False, debug=False)

    # all big params arrive pre-transposed, bf16, from the host
    xT_e = nc.declare_dram_parameter("xT", [D, N], bf16, isOutput=False)
    wT_e = {
        nm: nc.declare_dram_parameter(nm, [D, D], bf16, isOutput=False)
        for nm in ("WqT", "WkT", "WvT", "WoT")
    }
    a3_e = nc.declare_dram_parameter("A3T", [D, 96], bf16, isOutput=False)
    b3_e = nc.declare_dram_parameter("B3T", [112, D], bf16, isOutput=False)
    aT_e = {
        nm: nc.declare_dram_parameter(nm, [D, R], bf16, isOutput=False)
        for nm in ("AoT",)
    }
    bT_e = {
        nm: nc.declare_dram_parameter(nm, [49, D], bf16, isOutput=False)
        for nm in ("BoT",)
    }
    bm_e = nc.declare_dram_parameter("BM", [33, P], bf16, isOutput=False)
    out_e = nc.declare_dram_parameter("out", [N, D], bf16, isOutput=True)

    with tile.TileContext(nc) as tc:
        with (
            tc.tile_pool(name="wpool", bufs=1) as wpool,
            tc.tile_pool(name="stage", bufs=2) as stage,
            tc.tile_pool(name="ps", bufs=1, space="PSUM") as ps,
        ):
            qs = [nc.sync, nc.scalar, nc.gpsimd]
            xqs = qs

            # dummy for warm-up matmuls (must be initialized for the sim)
            wdummy = wpool.tile([P, P], bf16, tag="wdummy")
            nc.vector.memset(wdummy[:], 0.0)

            # ---- load pre-transposed tensors straight into SBUF ----
            T = {}
            aT = {}
            bT = {}
            qi = 0

            def load_big(nm, ext, queues=None):
                nonlocal qi
                queues = queues or qs
                T[nm] = []
                for t in range(8):
                    tt = wpool.tile([P, D], bf16, tag=f"T_{nm}_{t}",
                                    name=f"T_{nm}_{t}")
                    queues[qi % len(queues)].dma_start(
                        out=tt[:], in_=ext[t * P:(t + 1) * P, :])
                    qi += 1
                    T[nm].append(tt)

            def load_a(nm):
                nonlocal qi
                key = nm[:2]
                aT[key] = []
                for t in range(8):
                    tt = wpool.tile([P, R], bf16, tag=f"aT_{nm}_{t}",
                                    name=f"aT_{nm}_{t}")
                    qs[qi % 3].dma_start(out=tt[:],
                                         in_=aT_e[nm][t * P:(t + 1) * P, :])
                    qi += 1
                    aT[key].append(tt)

            def load_b(nm):
                nonlocal qi
                tt = wpool.tile([49, D], bf16, tag=f"bT_{nm}")
                qs[qi % 3].dma_start(out=tt[:], in_=bT_e[nm][:, :])
                qi += 1
                bT[nm[:2]] = tt

            a3 = []
            for t in range(8):
                tt = wpool.tile([P, 96], bf16, tag=f"a3_{t}",
                                name=f"a3_{t}")
                qs[qi % 3].dma_start(out=tt[:],
                                     in_=a3_e[t * P:(t + 1) * P, :])
                qi += 1
                a3.append(tt)
            b3 = wpool.tile([112, D], bf16, tag="b3")
            qs[qi % 3].dma_start(out=b3[:], in_=b3_e[:, :])
            qi += 1
            load_big("x", xT_e, queues=xqs)

            # ---- PE warm-up: ungated burst spans the DMA head, then
            # per-x-tile gated batches keep HAM at 8/8 until real work ----
            wps = ps.tile([P, F], f32, tag="pvpsum", bufs=2)
            for _ in range(120):
                nc.tensor.matmul(wps[:, 0:P], wdummy[:], wdummy[:],
                                 start=True, stop=True)
            for t in range(8):
                nbatch = (28, 32, 32, 32, 28, 24, 20, 16)[t]
                for _ in range(nbatch):
                    nc.tensor.matmul(wps[:, 0:P], wdummy[:],
                                     T["x"][t][:, 0:P], start=True, stop=True)

            load_big("Wv", wT_e["WvT"])
            load_big("Wq", wT_e["WqT"])
            load_big("Wk", wT_e["WkT"])
            load_big("Wo", wT_e["WoT"])
            load_a("AoT")
            load_b("BoT")
            onesb = wpool.tile([P, HD], bf16, tag="onesb")
            nc.vector.memset(onesb[:], 1.0)
            # block mask [2,128]: row 0 selects out partitions 0-63,
            # row 1 selects 64-127 -- one K=2 matmul broadcasts two
            # reciprocal rows to the two head-halves of a psum tile


            # ---- lora intermediates, q/k/v packed at 32-aligned rows
            # (host ships A3T/B3T with Aq@0, Ak@32, Av@64, Av-dup@96) ----
            tsb3 = []
            for nh in range(2):
                ns = slice(nh * F, (nh + 1) * F)
                pt = ps.tile([96, F], f32, tag="tpsum", bufs=1)
                for kt in range(8):
                    nc.tensor.matmul(pt[:], a3[kt][:], T["x"][kt][:, ns],
                                     start=(kt == 0), stop=(kt == 7))
                t_s = stage.tile([112, F], bf16, tag="tsb", bufs=2,
                                 name=f"tsb3_{nh}")
                nc.vector.tensor_scalar_mul(t_s[0:96, :], pt[:], SCALING)
                nc.vector.tensor_scalar_mul(t_s[96:112, :], pt[64:80, :],
                                            SCALING)
                tsb3.append(t_s)

            # ---- v natural, per-head layout [v_h | 1], with the dt=0
            # projection woven in so attention starts immediately after ----
            qks = {}

            def proj_gen(dt):
                qk = {}
                for nm, wnm, bnm, scl in (("q", "Wq", "Bq", SCALE),
                                          ("k", "Wk", "Bk", None)):
                    dst = wpool.tile([P, D], bf16, tag=f"{nm}T",
                                     bufs=3, name=f"{nm}T_{dt}")
                    qk[nm] = dst
                    for nh in range(2):
                        ns = slice(nh * F, (nh + 1) * F)
                        pq = ps.tile([P, F], f32, tag="projpsum", bufs=1)
                        for kt in range(8):
                            nc.tensor.matmul(
                                pq[:], T[wnm][kt][:, dt * P:(dt + 1) * P],
                                T["x"][kt][:, ns],
                                start=(kt == 0), stop=False)
                            yield
                        ro3 = 0 if nm == "q" else 32
                        nc.tensor.matmul(pq[:],
                                         b3[ro3:ro3 + R,
                                            dt * P:(dt + 1) * P],
                                         tsb3[nh][ro3:ro3 + R, :],
                                         start=False, stop=True)
                        yield
                        if scl is None:
                            nc.vector.tensor_copy(dst[:, ns], pq[:])
                        else:
                            nc.vector.tensor_scalar_mul(dst[:, ns],
                                                        pq[:], scl)
                        yield
                qks[dt] = qk

            VW = H * (HD + 1)  # 1040
            v_sb = [wpool.tile([P, VW], bf16, tag=f"v_{t}",
                               name=f"v_{t}") for t in range(8)]
            g0 = proj_gen(0)
            for nt in range(8):
                vr = v_sb[nt][:].rearrange("p (h c) -> p h c", c=HD + 1)
                pvs = []
                for dh in range(2):
                    ds = slice(dh * F, (dh + 1) * F)
                    pv = ps.tile([P, F], f32,
                                 tag=("spair" if dh == 0 else "pvpsum"),
                                 bufs=2)
                    pvs.append(pv)
                    for kt in range(8):
                        nc.tensor.matmul(
                            pv[:], T["x"][kt][:, nt * P:(nt + 1) * P],
                            T["Wv"][kt][:, ds], start=(kt == 0), stop=False)
                    for _ in range(3):
                        next(g0, None)
                # v-lora pair: tiles (64,0) and (96,0), concurrent
                nc.tensor.matmul(
                    pvs[0][:],
                    tsb3[nt // 4][64:80, (nt % 4) * P:(nt % 4 + 1) * P],
                    b3[64:80, 0:F], start=False, stop=True)
                nc.tensor.matmul(
                    pvs[1][:],
                    tsb3[nt // 4][96:112, (nt % 4) * P:(nt % 4 + 1) * P],
                    b3[96:112, F:2 * F], start=False, stop=True,
                    tile_position=(96, 0))
                for dh in range(2):
                    pvr = pvs[dh][:].rearrange("p (h c) -> p h c", c=HD)
                    nc.vector.tensor_copy(vr[:, dh * 8:(dh + 1) * 8, 0:HD],
                                          pvr[:])
                    for _ in range(2):
                        next(g0, None)
                nc.vector.memset(vr[:, :, HD:HD + 1], 1.0)
            for _ in g0:
                pass

            # ---- per dout-tile: qT, kT, then its 2 heads' attention.
            # The NEXT tile's projection matmuls are woven into the
            # attention inner loop (generator) so the PE stays dense
            # while ACT runs the exps -- keeps HAM at K=8/8. ----
            attnT = [wpool.tile([P, D], bf16, tag=f"attnT_{t}",
                                name=f"attnT_{t}") for t in range(8)]
            for dt in range(8):
                g = proj_gen(dt + 1) if dt < 7 else iter(())
                h0 = 2 * dt
                qt = qks[dt]["q"]
                ktt = qks[dt]["k"]
                for nh in range(2):
                    ns = slice(nh * F, (nh + 1) * F)
                    po = {}
                    for h in (h0, h0 + 1):
                        po[h] = ps.tile([HD + 1, F], f32, tag="pvpsum",
                                        bufs=2, name=f"po_{h}_{nh}")
                    for mt in range(8):
                        spair = ps.tile([P, 2 * F], f32, tag="spair",
                                        bufs=2)
                        for hi, h in enumerate((h0, h0 + 1)):
                            ro = (h % 2) * HD
                            m0 = mt * P
                            nc.tensor.matmul(
                                spair[:, hi * F:(hi + 1) * F],
                                ktt[ro:ro + HD, m0:m0 + P],
                                qt[ro:ro + HD, ns], start=True, stop=True)
                        pte = stage.tile([P, 2 * F], bf16, tag="pt", bufs=3)
                        nc.scalar.activation(pte[:], spair[:], Exp)
                        with tc.high_priority(offset=-40):
                            for hi, h in enumerate((h0, h0 + 1)):
                                nc.tensor.matmul(
                                    po[h][:],
                                    v_sb[mt][:,
                                             h * (HD + 1):(h + 1) * (HD + 1)],
                                    pte[:, hi * F:(hi + 1) * F],
                                    start=(mt == 0), stop=(mt == 7))
                        for _ in range(3):
                            next(g, None)
                    # normalize (baseline chain, bf16 broadcast):
                    # oah copy frees po; bf16 denom row -> K=1 broadcast
                    # (1 HW matmul vs 2 for fp32); recip fp32; multiply
                    for hi, h in enumerate((h0, h0 + 1)):
                        ro = (h % 2) * HD
                        oah = stage.tile([HD + 1, F], f32, tag="oah",
                                         bufs=3)
                        nc.vector.tensor_copy(oah[:], po[h][:])
                        dnb = stage.tile([1, F], bf16, tag="dnb", bufs=3)
                        nc.vector.tensor_copy(dnb[:], oah[HD:HD + 1, :])
                        pb = ps.tile([HD, F], f32, tag="tpsum", bufs=1)
                        nc.tensor.matmul(pb[:], onesb[0:1, 0:HD], dnb[:],
                                         start=True, stop=True)
                        pbs = stage.tile([HD, F], f32, tag="pbs", bufs=3)
                        nc.vector.reciprocal_approx_fast(pbs[:], pb[:])
                        ast = stage.tile([HD, F], bf16, tag="ast", bufs=3)
                        nc.vector.tensor_mul(ast[:], oah[0:HD, :], pbs[:])
                        nc.sync.dma_start(out=attnT[dt][ro:ro + HD, ns],
                                          in_=ast[:])
                        for _ in range(2):
                            next(g, None)
                for _ in g:
                    pass

            # ---- output projection ----
            # to rows: 0-15 Ao-lora (nh cols), 16 ones (bias row),
            # 32-48 duplicate so Bo-lora pairs run as concurrent tiles
            to = wpool.tile([49, D], bf16, tag="toT")
            nc.vector.memset(to[:], 1.0)
            pt_o = ps.tile([48, F], f32, tag="tpsum", bufs=1)
            for kt in range(8):
                nc.tensor.matmul(pt_o[0:R, :], aT["Ao"][kt][:],
                                 attnT[kt][:, 0:F],
                                 start=(kt == 0), stop=(kt == 7))
                nc.tensor.matmul(pt_o[32:32 + R, :], aT["Ao"][kt][:],
                                 attnT[kt][:, F:2 * F],
                                 start=(kt == 0), stop=(kt == 7))
            nc.vector.tensor_scalar_mul(to[0:R, 0:F], pt_o[0:R, :], SCALING)
            nc.vector.tensor_scalar_mul(to[0:R, F:2 * F],
                                        pt_o[32:32 + R, :], SCALING)
            nc.vector.tensor_copy(to[32:32 + R, 0:F], to[0:R, 0:F])
            nc.vector.tensor_copy(to[32:32 + R, F:2 * F], to[0:R, F:2 * F])
            for nt in range(8):
                nslc = slice(nt * P, (nt + 1) * P)
                pfs = []
                for dh in range(2):
                    ds = slice(dh * F, (dh + 1) * F)
                    pf = ps.tile([P, F], f32,
                                 tag=("spair" if dh == 0 else "pvpsum"),
                                 bufs=2)
                    pfs.append(pf)
                    for kt in range(8):
                        nc.tensor.matmul(pf[:], attnT[kt][:, nslc],
                                         T["Wo"][kt][:, ds],
                                         start=(kt == 0), stop=False)
                # Bo-lora pair: tiles (0,0) and (32,0), concurrent
                nc.tensor.matmul(pfs[0][:], to[0:R + 1, nslc],
                                 bT["Bo"][0:R + 1, 0:F],
                                 start=False, stop=True)
                nc.tensor.matmul(pfs[1][:], to[32:33 + R, nslc],
                                 bT["Bo"][32:33 + R, F:2 * F],
                                 start=False, stop=True)
                for dh in range(2):
                    ds = slice(dh * F, (dh + 1) * F)
                    osb = stage.tile([P, F], bf16, tag="osb")
                    nc.vector.tensor_copy(osb[:], pfs[dh][:])
                    nc.sync.dma_start(out=out_e[nslc, ds], in_=osb[:])
    nc.compile()
    return nc


def _get_nc():
    if "nc" not in _CACHE:
        _CACHE["nc"] = _build()
    return _CACHE["nc"]


def _prep_shared(inputs):
    def tb(a):  # transpose + bf16, contiguous
        return np.ascontiguousarray(np.asarray(a, np.float32).T.astype(BF16))

    shared = {}
    for nm in ("Wq", "Wk", "Wv", "Wo", "Ao"):
        shared[nm + "T"] = tb(inputs[nm])
    boa = np.zeros((49, D), np.float32)
    boa[0:R] = np.asarray(inputs["Bo"], np.float32).T
    boa[R] = np.asarray(inputs["bo"], np.float32)
    boa[32:32 + R] = boa[0:R]
    boa[32 + R] = boa[R]
    shared["BoT"] = np.ascontiguousarray(boa.astype(BF16))
    a3 = np.zeros((D, 96), np.float32)
    b3 = np.zeros((112, D), np.float32)
    for j, nm in enumerate(("q", "k", "v")):
        a3[:, 32 * j:32 * j + R] = np.asarray(inputs["A" + nm], np.float32).T
        b3[32 * j:32 * j + R, :] = np.asarray(inputs["B" + nm], np.float32).T
    b3[96:96 + R] = b3[64:64 + R]
    shared["A3T"] = np.ascontiguousarray(a3.astype(BF16))
    bm = np.zeros((33, P), np.float32)
    bm[0, 0:HD] = 1.0
    bm[32, HD:P] = 1.0
    shared["BM"] = np.ascontiguousarray(bm.astype(BF16))
    shared["B3T"] = np.ascontiguousarray(b3.astype(BF16))
    return shared


def kernel(**inputs):
    from concourse import bass_utils

    nc = _get_nc()
    shared = _prep_shared(inputs)
    x = np.asarray(inputs["x"], np.float32)
    in_maps = []
    for i in range(NCORES):
        m = dict(shared)
        m["xT"] = np.ascontiguousarray(x[i].T.astype(BF16))
        in_maps.append(m)
    res = bass_utils.run_bass_kernel_spmd(nc, in_maps,
                                          core_ids=list(range(NCORES)))
    return np.stack([np.asarray(res.results[i]["out"]).astype(np.float32)
                     for i in range(NCORES)], axis=0)


# revision 20
# speedup vs baseline: 1.2460x; 1.0089x over previous
"""LoRA multi-head attention on 8 TRN2 NeuronCores.

Sharding: data-parallel over batch (B=8 -> 1 batch element per core),
weights replicated, no collectives.

Host side (in kernel()): inputs are cast to bf16 and pre-transposed so
the device reads exactly the layouts the TensorEngine needs (the
contraction dim on partitions). LoRA B matrices are packed at 32-aligned
partition offsets (with duplicated rows where two matmuls must run
concurrently as PE row-tiles).

Device side per core, all bf16 with fp32 PSUM accumulation:
  qT = (WqT.T @ xT + BqT.T (AqT.T xT) / 16) / 8     [dout, n]
  kT likewise; v natural [n, dout] via (xT.T @ WvT), stored per-head
  with a ones column ([v_h | 1]) so PV also yields softmax denoms.
  Per head: S^T = kT_h.T qT_h -> exp (no max-sub; |s|=O(4)) -> PV;
  normalize via bf16 reciprocal-row broadcast matmuls (two concurrent
  PE tiles) instead of fp32 ones-outer-products.
  out = attnT.T @ WoT + lora + bo (bias via K=1 ones matmul).

PE warm-up runs ungated during the input DMA window (HAM reaches 8/8
before real work), paced by per-x-tile gated batches.
"""

import sys

if "/opt/trn_rl_repo" not in sys.path:
    sys.path.insert(0, "/opt/trn_rl_repo")

import numpy as np
import ml_dtypes

BF16 = ml_dtypes.bfloat16

N = 1024  # tokens
D = 1024  # model dim
H = 16    # heads
HD = 64   # head dim
R = 16    # lora rank
P = 128   # partitions
F = 512   # psum free-dim tile
NCORES = 8
SCALING = 1.0 / 16.0  # lora alpha/rank
SCALE = HD ** -0.5

_CACHE = {}


def _build():
    import concourse.bacc as bacc
    import concourse.mybir as mybir
    import concourse.tile as tile

    f32 = mybir.dt.float32
    bf16 = mybir.dt.bfloat16
    Exp = mybir.ActivationFunctionType.Exp

    nc = bacc.Bacc("TRN2", target_bir_lowering=False, debug=False)

    # all big params arrive pre-transposed, bf16, from the host
    xT_e = nc.declare_dram_parameter("xT", [D, N], bf16, isOutput=False)
    wT_e = {
        nm: nc.declare_dram_parameter(nm, [D, D], bf16, isOutput=False)
        for nm in ("WqT", "WkT", "WvT", "WoT")
    }
    a3_e = nc.declare_dram_parameter("A3T", [D, 96], bf16, isOutput=False)
    b3_e = nc.declare_dram_parameter("B3T", [112, D], bf16, isOutput=False)
    aT_e = {
        nm: nc.declare_dram_parameter(nm, [D, R], bf16, isOutput=False)
        for nm in ("AoT",)
    }
    bT_e = {
        nm: nc.declare_dram_parameter(nm, [49, D], bf16, isOutput=False)
        for nm in ("BoT",)
    }
    bm_e = nc.declare_dram_parameter("BM", [33, P], bf16, isOutput=False)
    out_e = nc.declare_dram_parameter("out", [N, D], bf16, isOutput=True)

    with tile.TileContext(nc) as tc:
        with (
            tc.tile_pool(name="wpool", bufs=1) as wpool,
            tc.tile_pool(name="stage", bufs=2) as stage,
            tc.tile_pool(name="ps", bufs=1, space="PSUM") as ps,
        ):
            qs = [nc.sync, nc.scalar, nc.gpsimd]
            xqs = qs

            # dummy for warm-up matmuls (must be initialized for the sim)
            wdummy = wpool.tile([P, P], bf16, tag="wdummy")
            nc.vector.memset(wdummy[:], 0.0)

            # ---- load pre-transposed tensors straight into SBUF ----
            T = {}
            aT = {}
            bT = {}
            qi = 0

            def load_big(nm, ext, queues=None):
                nonlocal qi
                queues = queues or qs
                T[nm] = []
                for t in range(8):
                    tt = wpool.tile([P, D], bf16, tag=f"T_{nm}_{t}",
                                    name=f"T_{nm}_{t}")
                    queues[qi % len(queues)].dma_start(
                        out=tt[:], in_=ext[t * P:(t + 1) * P, :])
                    qi += 1
                    T[nm].append(tt)

            def load_a(nm):
                nonlocal qi
                key = nm[:2]
                aT[key] = []
                for t in range(8):
                    tt = wpool.tile([P, R], bf16, tag=f"aT_{nm}_{t}",
                                    name=f"aT_{nm}_{t}")
                    qs[qi % 3].dma_start(out=tt[:],
                                         in_=aT_e[nm][t * P:(t + 1) * P, :])
                    qi += 1
                    aT[key].append(tt)

            def load_b(nm):
                nonlocal qi
                tt = wpool.tile([49, D], bf16, tag=f"bT_{nm}")
                qs[qi % 3].dma_start(out=tt[:], in_=bT_e[nm][:, :])
                qi += 1
                bT[nm[:2]] = tt

            a3 = []
            for t in range(8):
                tt = wpool.tile([P, 96], bf16, tag=f"a3_{t}",
                                name=f"a3_{t}")
                qs[qi % 3].dma_start(out=tt[:],
                                     in_=a3_e[t * P:(t + 1) * P, :])
                qi += 1
                a3.append(tt)
            b3 = wpool.tile([112, D], bf16, tag="b3")
            qs[qi % 3].dma_start(out=b3[:], in_=b3_e[:, :])
            qi += 1
            load_big("x", xT_e, queues=xqs)

            # ---- PE warm-up: ungated burst spans the DMA head, then
            # per-x-tile gated batches keep HAM at 8/8 until real work ----
            wps = ps.tile([P, F], f32, tag="pvpsum", bufs=2)
            for _ in range(120):
                nc.tensor.matmul(wps[:, 0:P], wdummy[:], wdummy[:],
                                 start=True, stop=True)
            for t in range(8):
                nbatch = (28, 32, 32, 32, 28, 24, 20, 16)[t]
                for _ in range(nbatch):
                    nc.tensor.matmul(wps[:, 0:P], wdummy[:],
                                     T["x"][t][:, 0:P], start=True, stop=True)

            load_big("Wv", wT_e["WvT"])
            load_big("Wq", wT_e["WqT"])
            load_big("Wk", wT_e["WkT"])
            load_big("Wo", wT_e["WoT"])
            load_a("AoT")
            load_b("BoT")
            onesb = wpool.tile([P, HD], bf16, tag="onesb")
            nc.vector.memset(onesb[:], 1.0)
            # block mask [2,128]: row 0 selects out partitions 0-63,
            # row 1 selects 64-127 -- one K=2 matmul broadcasts two
            # reciprocal rows to the two head-halves of a psum tile


            # ---- lora intermediates, q/k/v packed at 32-aligned rows
            # (host ships A3T/B3T with Aq@0, Ak@32, Av@64, Av-dup@96) ----
            tsb3 = []
            for nh in range(2):
                ns = slice(nh * F, (nh + 1) * F)
                pt = ps.tile([96, F], f32, tag="tpsum", bufs=1)
                for kt in range(8):
                    nc.tensor.matmul(pt[:], a3[kt][:], T["x"][kt][:, ns],
                                     start=(kt == 0), stop=(kt == 7))
                t_s = stage.tile([112, F], bf16, tag="tsb", bufs=2,
                                 name=f"tsb3_{nh}")
                nc.vector.tensor_scalar_mul(t_s[0:96, :], pt[:], SCALING)
                nc.vector.tensor_scalar_mul(t_s[96:112, :], pt[64:80, :],
                                            SCALING)
                tsb3.append(t_s)

            # ---- v natural, per-head layout [v_h | 1], with the dt=0
            # projection woven in so attention starts immediately after ----
            qks = {}

            def proj_gen(dt):
                qk = {}
                for nm, wnm, bnm, scl in (("q", "Wq", "Bq", SCALE),
                                          ("k", "Wk", "Bk", None)):
                    dst = wpool.tile([P, D], bf16, tag=f"{nm}T",
                                     bufs=3, name=f"{nm}T_{dt}")
                    qk[nm] = dst
                    for nh in range(2):
                        ns = slice(nh * F, (nh + 1) * F)
                        pq = ps.tile([P, F], f32, tag="projpsum", bufs=1)
                        for kt in range(8):
                            nc.tensor.matmul(
                                pq[:], T[wnm][kt][:, dt * P:(dt + 1) * P],
                                T["x"][kt][:, ns],
                                start=(kt == 0), stop=False)
                            yield
                        ro3 = 0 if nm == "q" else 32
                        nc.tensor.matmul(pq[:],
                                         b3[ro3:ro3 + R,
                                            dt * P:(dt + 1) * P],
                                         tsb3[nh][ro3:ro3 + R, :],
                                         start=False, stop=True)
                        yield
                        if scl is None:
                            nc.vector.tensor_copy(dst[:, ns], pq[:])
                        else:
                            nc.vector.tensor_scalar_mul(dst[:, ns],
                                                        pq[:], scl)
                        yield
                qks[dt] = qk

            VW = H * (HD + 1)  # 1040
            v_sb = [wpool.tile([P, VW], bf16, tag=f"v_{t}",
                               name=f"v_{t}") for t in range(8)]
            g0 = proj_gen(0)
            for nt in range(8):
                vr = v_sb[nt][:].rearrange("p (h c) -> p h c", c=HD + 1)
                pvs = []
                for dh in range(2):
                    ds = slice(dh * F, (dh + 1) * F)
                    pv = ps.tile([P, F], f32,
                                 tag=("spair" if dh == 0 else "pvpsum"),
                                 bufs=2)
                    pvs.append(pv)
                    for kt in range(8):
                        nc.tensor.matmul(
                            pv[:], T["x"][kt][:, nt * P:(nt + 1) * P],
                            T["Wv"][kt][:, ds], start=(kt == 0), stop=False)
                    for _ in range(3):
                        next(g0, None)
                # v-lora pair: tiles (64,0) and (96,0), concurrent
                nc.tensor.matmul(
                    pvs[0][:],
                    tsb3[nt // 4][64:80, (nt % 4) * P:(nt % 4 + 1) * P],
                    b3[64:80, 0:F], start=False, stop=True)
                nc.tensor.matmul(
                    pvs[1][:],
                    tsb3[nt // 4][96:112, (nt % 4) * P:(nt % 4 + 1) * P],
                    b3[96:112, F:2 * F], start=False, stop=True,
                    tile_position=(96, 0))
                for dh in range(2):
                    pvr = pvs[dh][:].rearrange("p (h c) -> p h c", c=HD)
                    nc.vector.tensor_copy(vr[:, dh * 8:(dh + 1) * 8, 0:HD],
                                          pvr[:])
                    for _ in range(2):
                        next(g0, None)
                nc.vector.memset(vr[:, :, HD:HD + 1], 1.0)
            for _ in g0:
                pass

            # ---- per dout-tile: qT, kT, then its 2 heads' attention.
            # The NEXT tile's projection matmuls are woven into the
            # attention inner loop (generator) so the PE stays dense
            # while ACT runs the exps -- keeps HAM at K=8/8. ----
            attnT = [wpool.tile([P, D], bf16, tag=f"attnT_{t}",
                                name=f"attnT_{t}") for t in range(8)]
            for dt in range(8):
                g = proj_gen(dt + 1) if dt < 7 else iter(())
                h0 = 2 * dt
                qt = qks[dt]["q"]
                ktt = qks[dt]["k"]
                for nh in range(2):
                    ns = slice(nh * F, (nh + 1) * F)
                    po = {}
                    for h in (h0, h0 + 1):
                        po[h] = ps.tile([HD + 1, F], f32, tag="pvpsum",
                                        bufs=2, name=f"po_{h}_{nh}")
                    for mt in range(8):
                        spair = ps.tile([P, 2 * F], f32, tag="spair",
                                        bufs=2)
                        for hi, h in enumerate((h0, h0 + 1)):
                            ro = (h % 2) * HD
                            m0 = mt * P
                            nc.tensor.matmul(
                                spair[:, hi * F:(hi + 1) * F],
                                ktt[ro:ro + HD, m0:m0 + P],
                                qt[ro:ro + HD, ns], start=True, stop=True)
                        pte = stage.tile([P, 2 * F], bf16, tag="pt", bufs=3)
                        nc.scalar.activation(pte[:], spair[:], Exp)
                        with tc.high_priority(offset=-40):
                            for hi, h in enumerate((h0, h0 + 1)):
                                nc.tensor.matmul(
                                    po[h][:],
                                    v_sb[mt][:,
                                             h * (HD + 1):(h + 1) * (HD + 1)],
                                    pte[:, hi * F:(hi + 1) * F],
                                    start=(mt == 0), stop=(mt == 7))
                        for _ in range(3):
                            next(g, None)
                    # normalize (baseline chain, bf16 broadcast):
                    # oah copy frees po; bf16 denom row -> K=1 broadcast
                    # (1 HW matmul vs 2 for fp32); recip fp32; multiply
                    for hi, h in enumerate((h0, h0 + 1)):
                        ro = (h % 2) * HD
                        oah = stage.tile([HD + 1, F], f32, tag="oah",
                                         bufs=3)
                        nc.vector.tensor_copy(oah[:], po[h][:])
                        dnb = stage.tile([1, F], bf16, tag="dnb", bufs=3)
                        nc.vector.tensor_copy(dnb[:], oah[HD:HD + 1, :])
                        pb = ps.tile([HD, F], f32, tag="tpsum", bufs=1)
                        with tc.high_priority(offset=-40):
                            nc.tensor.matmul(pb[:], onesb[0:1, 0:HD],
                                             dnb[:], start=True, stop=True)
                        pbs = stage.tile([HD, F], f32, tag="pbs", bufs=3)
                        nc.vector.reciprocal_approx_fast(pbs[:], pb[:])
                        ast = stage.tile([HD, F], bf16, tag="ast", bufs=3)
                        nc.vector.tensor_mul(ast[:], oah[0:HD, :], pbs[:])
                        nc.sync.dma_start(out=attnT[dt][ro:ro + HD, ns],
                                          in_=ast[:])
                        for _ in range(2):
                            next(g, None)
                for _ in g:
                    pass

            # ---- output projection ----
            # to rows: 0-15 Ao-lora (nh cols), 16 ones (bias row),
            # 32-48 duplicate so Bo-lora pairs run as concurrent tiles
            to = wpool.tile([49, D], bf16, tag="toT")
            nc.vector.memset(to[:], 1.0)
            pt_o = ps.tile([48, F], f32, tag="tpsum", bufs=1)
            for kt in range(8):
                nc.tensor.matmul(pt_o[0:R, :], aT["Ao"][kt][:],
                                 attnT[kt][:, 0:F],
                                 start=(kt == 0), stop=(kt == 7))
                nc.tensor.matmul(pt_o[32:32 + R, :], aT["Ao"][kt][:],
                                 attnT[kt][:, F:2 * F],
                                 start=(kt == 0), stop=(kt == 7))
            nc.vector.tensor_scalar_mul(to[0:R, 0:F], pt_o[0:R, :], SCALING)
            nc.vector.tensor_scalar_mul(to[0:R, F:2 * F],
                                        pt_o[32:32 + R, :], SCALING)
            nc.vector.tensor_copy(to[32:32 + R, 0:F], to[0:R, 0:F])
            nc.vector.tensor_copy(to[32:32 + R, F:2 * F], to[0:R, F:2 * F])
            for nt in range(8):
                nslc = slice(nt * P, (nt + 1) * P)
                pfs = []
                for dh in range(2):
                    ds = slice(dh * F, (dh + 1) * F)
                    pf = ps.tile([P, F], f32,
                                 tag=("spair" if dh == 0 else "pvpsum"),
                                 bufs=2)
                    pfs.append(pf)
                    for kt in range(8):
                        nc.tensor.matmul(pf[:], attnT[kt][:, nslc],
                                         T["Wo"][kt][:, ds],
                                         start=(kt == 0), stop=False)
                # Bo-lora pair: tiles (0,0) and (32,0), concurrent
                nc.tensor.matmul(pfs[0][:], to[0:R + 1, nslc],
                                 bT["Bo"][0:R + 1, 0:F],
                                 start=False, stop=True)
                nc.tensor.matmul(pfs[1][:], to[32:33 + R, nslc],
                                 bT["Bo"][32:33 + R, F:2 * F],
                                 start=False, stop=True)
                for dh in range(2):
                    ds = slice(dh * F, (dh + 1) * F)
                    osb = stage.tile([P, F], bf16, tag="osb")
                    nc.vector.tensor_copy(osb[:], pfs[dh][:])
                    nc.sync.dma_start(out=out_e[nslc, ds], in_=osb[:])
    nc.compile()
    return nc


def _get_nc():
    if "nc" not in _CACHE:
        _CACHE["nc"] = _build()
    return _CACHE["nc"]


def _prep_shared(inputs):
    def tb(a):  # transpose + bf16, contiguous
        return np.ascontiguousarray(np.asarray(a, np.float32).T.astype(BF16))

    shared = {}
    for nm in ("Wq", "Wk", "Wv", "Wo", "Ao"):
        shared[nm + "T"] = tb(inputs[nm])
    boa = np.zeros((49, D), np.float32)
    boa[0:R] = np.asarray(inputs["Bo"], np.float32).T
    boa[R] = np.asarray(inputs["bo"], np.float32)
    boa[32:32 + R] = boa[0:R]
    boa[32 + R] = boa[R]
    shared["BoT"] = np.ascontiguousarray(boa.astype(BF16))
    a3 = np.zeros((D, 96), np.float32)
    b3 = np.zeros((112, D), np.float32)
    for j, nm in enumerate(("q", "k", "v")):
        a3[:, 32 * j:32 * j + R] = np.asarray(inputs["A" + nm], np.float32).T
        b3[32 * j:32 * j + R, :] = np.asarray(inputs["B" + nm], np.float32).T
    b3[96:96 + R] = b3[64:64 + R]
    shared["A3T"] = np.ascontiguousarray(a3.astype(BF16))
    bm = np.zeros((33, P), np.float32)
    bm[0, 0:HD] = 1.0
    bm[32, HD:P] = 1.0
    shared["BM"] = np.ascontiguousarray(bm.astype(BF16))
    shared["B3T"] = np.ascontiguousarray(b3.astype(BF16))
    return shared


def kernel(**inputs):
    from concourse import bass_utils

    nc = _get_nc()
    shared = _prep_shared(inputs)
    x = np.asarray(inputs["x"], np.float32)
    in_maps = []
    for i in range(NCORES):
        m = dict(shared)
        m["xT"] = np.ascontiguousarray(x[i].T.astype(BF16))
        in_maps.append(m)
    res = bass_utils.run_bass_kernel_spmd(nc, in_maps,
                                          core_ids=list(range(NCORES)))
    return np.stack([np.asarray(res.results[i]["out"]).astype(np.float32)
                     for i in range(NCORES)], axis=0)


# revision 21
# speedup vs baseline: 1.2514x; 1.0043x over previous
"""LoRA multi-head attention on 8 TRN2 NeuronCores.

Sharding: data-parallel over batch (B=8 -> 1 batch element per core),
weights replicated, no collectives.

Host side (in kernel()): inputs are cast to bf16 and pre-transposed so
the device reads exactly the layouts the TensorEngine needs (the
contraction dim on partitions). LoRA B matrices are packed at 32-aligned
partition offsets (with duplicated rows where two matmuls must run
concurrently as PE row-tiles).

Device side per core, all bf16 with fp32 PSUM accumulation:
  qT = (WqT.T @ xT + BqT.T (AqT.T xT) / 16) / 8     [dout, n]
  kT likewise; v natural [n, dout] via (xT.T @ WvT), stored per-head
  with a ones column ([v_h | 1]) so PV also yields softmax denoms.
  Per head: S^T = kT_h.T qT_h -> exp (no max-sub; |s|=O(4)) -> PV;
  normalize via bf16 reciprocal-row broadcast matmuls (two concurrent
  PE tiles) instead of fp32 ones-outer-products.
  out = attnT.T @ WoT + lora + bo (bias via K=1 ones matmul).

PE warm-up runs ungated during the input DMA window (HAM reaches 8/8
before real work), paced by per-x-tile gated batches.
"""

import sys

if "/opt/trn_rl_repo" not in sys.path:
    sys.path.insert(0, "/opt/trn_rl_repo")

import numpy as np
import ml_dtypes

BF16 = ml_dtypes.bfloat16

N = 1024  # tokens
D = 1024  # model dim
H = 16    # heads
HD = 64   # head dim
R = 16    # lora rank
P = 128   # partitions
F = 512   # psum free-dim tile
NCORES = 8
SCALING = 1.0 / 16.0  # lora alpha/rank
SCALE = HD ** -0.5

_CACHE = {}


def _build():
    import concourse.bacc as bacc
    import concourse.mybir as mybir
    import concourse.tile as tile

    f32 = mybir.dt.float32
    bf16 = mybir.dt.bfloat16
    Exp = mybir.ActivationFunctionType.Exp

    nc = bacc.Bacc("TRN2", target_bir_lowering=False, debug=False)

    # all big params arrive pre-transposed, bf16, from the host
    xT_e = nc.declare_dram_parameter("xT", [D, N], bf16, isOutput=False)
    wT_e = {
        nm: nc.declare_dram_parameter(nm, [D, D], bf16, isOutput=False)
        for nm in ("WqT", "WkT", "WvT", "WoT")
    }
    a3_e = nc.declare_dram_parameter("A3T", [D, 96], bf16, isOutput=False)
    b3_e = nc.declare_dram_parameter("B3T", [112, D], bf16, isOutput=False)
    aT_e = {
        nm: nc.declare_dram_parameter(nm, [D, R], bf16, isOutput=False)
        for nm in ("AoT",)
    }
    bT_e = {
        nm: nc.declare_dram_parameter(nm, [49, D], bf16, isOutput=False)
        for nm in ("BoT",)
    }
    bm_e = nc.declare_dram_parameter("BM", [33, P], bf16, isOutput=False)
    out_e = nc.declare_dram_parameter("out", [N, D], bf16, isOutput=True)

    with tile.TileContext(nc) as tc:
        with (
            tc.tile_pool(name="wpool", bufs=1) as wpool,
            tc.tile_pool(name="stage", bufs=2) as stage,
            tc.tile_pool(name="ps", bufs=1, space="PSUM") as ps,
        ):
            qs = [nc.sync, nc.scalar, nc.gpsimd]
            xqs = qs

            # dummy for warm-up matmuls (must be initialized for the sim)
            wdummy = wpool.tile([P, P], bf16, tag="wdummy")
            nc.vector.memset(wdummy[:], 0.0)

            # ---- load pre-transposed tensors straight into SBUF ----
            T = {}
            aT = {}
            bT = {}
            qi = 0

            def load_big(nm, ext, queues=None, split=False):
                nonlocal qi
                queues = queues or qs
                T[nm] = []
                for t in range(8):
                    tt = wpool.tile([P, D], bf16, tag=f"T_{nm}_{t}",
                                    name=f"T_{nm}_{t}")
                    if split:
                        h = P // 2
                        for u in range(2):
                            queues[qi % len(queues)].dma_start(
                                out=tt[u * h:(u + 1) * h, :],
                                in_=ext[t * P + u * h:t * P + (u + 1) * h, :])
                            qi += 1
                    else:
                        queues[qi % len(queues)].dma_start(
                            out=tt[:], in_=ext[t * P:(t + 1) * P, :])
                        qi += 1
                    T[nm].append(tt)

            def load_a(nm):
                nonlocal qi
                key = nm[:2]
                aT[key] = []
                for t in range(8):
                    tt = wpool.tile([P, R], bf16, tag=f"aT_{nm}_{t}",
                                    name=f"aT_{nm}_{t}")
                    qs[qi % 3].dma_start(out=tt[:],
                                         in_=aT_e[nm][t * P:(t + 1) * P, :])
                    qi += 1
                    aT[key].append(tt)

            def load_b(nm):
                nonlocal qi
                tt = wpool.tile([49, D], bf16, tag=f"bT_{nm}")
                qs[qi % 3].dma_start(out=tt[:], in_=bT_e[nm][:, :])
                qi += 1
                bT[nm[:2]] = tt

            a3 = []
            for t in range(8):
                tt = wpool.tile([P, 96], bf16, tag=f"a3_{t}",
                                name=f"a3_{t}")
                qs[qi % 3].dma_start(out=tt[:],
                                     in_=a3_e[t * P:(t + 1) * P, :])
                qi += 1
                a3.append(tt)
            b3 = wpool.tile([112, D], bf16, tag="b3")
            qs[qi % 3].dma_start(out=b3[:], in_=b3_e[:, :])
            qi += 1
            load_big("x", xT_e, queues=xqs, split=True)

            # ---- PE warm-up: ungated burst spans the DMA head, then
            # per-x-tile gated batches keep HAM at 8/8 until real work ----
            wps = ps.tile([P, F], f32, tag="pvpsum", bufs=2)
            for _ in range(120):
                nc.tensor.matmul(wps[:, 0:P], wdummy[:], wdummy[:],
                                 start=True, stop=True)
            for t in range(8):
                nbatch = (24, 24, 24, 24, 20, 20, 16, 12)[t]
                for _ in range(nbatch):
                    nc.tensor.matmul(wps[:, 0:P], wdummy[:],
                                     T["x"][t][:, 0:P], start=True, stop=True)

            load_big("Wv", wT_e["WvT"])
            load_big("Wq", wT_e["WqT"])
            load_big("Wk", wT_e["WkT"])
            load_big("Wo", wT_e["WoT"])
            load_a("AoT")
            load_b("BoT")
            onesb = wpool.tile([P, HD], bf16, tag="onesb")
            nc.vector.memset(onesb[:], 1.0)
            # block mask [2,128]: row 0 selects out partitions 0-63,
            # row 1 selects 64-127 -- one K=2 matmul broadcasts two
            # reciprocal rows to the two head-halves of a psum tile


            # ---- lora intermediates, q/k/v packed at 32-aligned rows
            # (host ships A3T/B3T with Aq@0, Ak@32, Av@64, Av-dup@96) ----
            tsb3 = []
            for nh in range(2):
                ns = slice(nh * F, (nh + 1) * F)
                pt = ps.tile([96, F], f32, tag="tpsum", bufs=1)
                for kt in range(8):
                    nc.tensor.matmul(pt[:], a3[kt][:], T["x"][kt][:, ns],
                                     start=(kt == 0), stop=(kt == 7))
                t_s = stage.tile([112, F], bf16, tag="tsb", bufs=2,
                                 name=f"tsb3_{nh}")
                nc.vector.tensor_scalar_mul(t_s[0:96, :], pt[:], SCALING)
                nc.vector.tensor_scalar_mul(t_s[96:112, :], pt[64:80, :],
                                            SCALING)
                tsb3.append(t_s)

            # ---- v natural, per-head layout [v_h | 1], with the dt=0
            # projection woven in so attention starts immediately after ----
            qks = {}

            def proj_gen(dt):
                qk = {}
                for nm, wnm, bnm, scl in (("q", "Wq", "Bq", SCALE),
                                          ("k", "Wk", "Bk", None)):
                    dst = wpool.tile([P, D], bf16, tag=f"{nm}T",
                                     bufs=3, name=f"{nm}T_{dt}")
                    qk[nm] = dst
                    for nh in range(2):
                        ns = slice(nh * F, (nh + 1) * F)
                        pq = ps.tile([P, F], f32, tag="projpsum", bufs=1)
                        for kt in range(8):
                            nc.tensor.matmul(
                                pq[:], T[wnm][kt][:, dt * P:(dt + 1) * P],
                                T["x"][kt][:, ns],
                                start=(kt == 0), stop=False)
                            yield
                        ro3 = 0 if nm == "q" else 32
                        nc.tensor.matmul(pq[:],
                                         b3[ro3:ro3 + R,
                                            dt * P:(dt + 1) * P],
                                         tsb3[nh][ro3:ro3 + R, :],
                                         start=False, stop=True)
                        yield
                        if scl is None:
                            nc.vector.tensor_copy(dst[:, ns], pq[:])
                        else:
                            nc.vector.tensor_scalar_mul(dst[:, ns],
                                                        pq[:], scl)
                        yield
                qks[dt] = qk

            VW = H * (HD + 1)  # 1040
            v_sb = [wpool.tile([P, VW], bf16, tag=f"v_{t}",
                               name=f"v_{t}") for t in range(8)]
            g0 = proj_gen(0)
            for nt in range(8):
                vr = v_sb[nt][:].rearrange("p (h c) -> p h c", c=HD + 1)
                pvs = []
                for dh in range(2):
                    ds = slice(dh * F, (dh + 1) * F)
                    pv = ps.tile([P, F], f32,
                                 tag=("spair" if dh == 0 else "pvpsum"),
                                 bufs=2)
                    pvs.append(pv)
                    for kt in range(8):
                        nc.tensor.matmul(
                            pv[:], T["x"][kt][:, nt * P:(nt + 1) * P],
                            T["Wv"][kt][:, ds], start=(kt == 0), stop=False)
                    for _ in range(3):
                        next(g0, None)
                # v-lora pair: tiles (64,0) and (96,0), concurrent
                nc.tensor.matmul(
                    pvs[0][:],
                    tsb3[nt // 4][64:80, (nt % 4) * P:(nt % 4 + 1) * P],
                    b3[64:80, 0:F], start=False, stop=True)
                nc.tensor.matmul(
                    pvs[1][:],
                    tsb3[nt // 4][96:112, (nt % 4) * P:(nt % 4 + 1) * P],
                    b3[96:112, F:2 * F], start=False, stop=True,
                    tile_position=(96, 0))
                for dh in range(2):
                    pvr = pvs[dh][:].rearrange("p (h c) -> p h c", c=HD)
                    nc.scalar.copy(vr[:, dh * 8:(dh + 1) * 8, 0:HD],
                                   pvr[:])
                    for _ in range(2):
                        next(g0, None)
                nc.vector.memset(vr[:, :, HD:HD + 1], 1.0)
            for _ in g0:
                pass

            # ---- per dout-tile: qT, kT, then its 2 heads' attention.
            # The NEXT tile's projection matmuls are woven into the
            # attention inner loop (generator) so the PE stays dense
            # while ACT runs the exps -- keeps HAM at K=8/8. ----
            attnT = [wpool.tile([P, D], bf16, tag=f"attnT_{t}",
                                name=f"attnT_{t}") for t in range(8)]
            for dt in range(8):
                g = proj_gen(dt + 1) if dt < 7 else iter(())
                h0 = 2 * dt
                qt = qks[dt]["q"]
                ktt = qks[dt]["k"]
                for nh in range(2):
                    ns = slice(nh * F, (nh + 1) * F)
                    po = {}
                    for h in (h0, h0 + 1):
                        po[h] = ps.tile([HD + 1, F], f32, tag="pvpsum",
                                        bufs=2, name=f"po_{h}_{nh}")
                    for mt in range(8):
                        spair = ps.tile([P, 2 * F], f32, tag="spair",
                                        bufs=2)
                        for hi, h in enumerate((h0, h0 + 1)):
                            ro = (h % 2) * HD
                            m0 = mt * P
                            nc.tensor.matmul(
                                spair[:, hi * F:(hi + 1) * F],
                                ktt[ro:ro + HD, m0:m0 + P],
                                qt[ro:ro + HD, ns], start=True, stop=True)
                        pte = stage.tile([P, 2 * F], bf16, tag="pt", bufs=3)
                        nc.scalar.activation(pte[:], spair[:], Exp)
                        with tc.high_priority(offset=-40):
                            for hi, h in enumerate((h0, h0 + 1)):
                                nc.tensor.matmul(
                                    po[h][:],
                                    v_sb[mt][:,
                                             h * (HD + 1):(h + 1) * (HD + 1)],
                                    pte[:, hi * F:(hi + 1) * F],
                                    start=(mt == 0), stop=(mt == 7))
                        for _ in range(3):
                            next(g, None)
                    # normalize (baseline chain, bf16 broadcast):
                    # oah copy frees po; bf16 denom row -> K=1 broadcast
                    # (1 HW matmul vs 2 for fp32); recip fp32; multiply
                    for hi, h in enumerate((h0, h0 + 1)):
                        ro = (h % 2) * HD
                        oah = stage.tile([HD + 1, F], f32, tag="oah",
                                         bufs=3)
                        nc.vector.tensor_copy(oah[:], po[h][:])
                        dnb = stage.tile([1, F], bf16, tag="dnb", bufs=3)
                        nc.vector.tensor_copy(dnb[:], oah[HD:HD + 1, :])
                        pb = ps.tile([HD, F], f32, tag="tpsum", bufs=1)
                        with tc.high_priority(offset=-40):
                            nc.tensor.matmul(pb[:], onesb[0:1, 0:HD],
                                             dnb[:], start=True, stop=True)
                        pbs = stage.tile([HD, F], f32, tag="pbs", bufs=3)
                        nc.vector.reciprocal_approx_fast(pbs[:], pb[:])
                        ast = stage.tile([HD, F], bf16, tag="ast", bufs=3)
                        nc.vector.tensor_mul(ast[:], oah[0:HD, :], pbs[:])
                        nc.sync.dma_start(out=attnT[dt][ro:ro + HD, ns],
                                          in_=ast[:])
                        for _ in range(2):
                            next(g, None)
                for _ in g:
                    pass

            # ---- output projection ----
            # to rows: 0-15 Ao-lora (nh cols), 16 ones (bias row),
            # 32-48 duplicate so Bo-lora pairs run as concurrent tiles
            to = wpool.tile([49, D], bf16, tag="toT")
            nc.vector.memset(to[:], 1.0)
            pt_o = ps.tile([48, F], f32, tag="tpsum", bufs=1)
            for kt in range(8):
                nc.tensor.matmul(pt_o[0:R, :], aT["Ao"][kt][:],
                                 attnT[kt][:, 0:F],
                                 start=(kt == 0), stop=(kt == 7))
                nc.tensor.matmul(pt_o[32:32 + R, :], aT["Ao"][kt][:],
                                 attnT[kt][:, F:2 * F],
                                 start=(kt == 0), stop=(kt == 7))
            nc.vector.tensor_scalar_mul(to[0:R, 0:F], pt_o[0:R, :], SCALING)
            nc.vector.tensor_scalar_mul(to[0:R, F:2 * F],
                                        pt_o[32:32 + R, :], SCALING)
            nc.vector.tensor_copy(to[32:32 + R, 0:F], to[0:R, 0:F])
            nc.vector.tensor_copy(to[32:32 + R, F:2 * F], to[0:R, F:2 * F])
            for nt in range(8):
                nslc = slice(nt * P, (nt + 1) * P)
                pfs = []
                for dh in range(2):
                    ds = slice(dh * F, (dh + 1) * F)
                    pf = ps.tile([P, F], f32,
                                 tag=("spair" if dh == 0 else "pvpsum"),
                                 bufs=2)
                    pfs.append(pf)
                    for kt in range(8):
                        nc.tensor.matmul(pf[:], attnT[kt][:, nslc],
                                         T["Wo"][kt][:, ds],
                                         start=(kt == 0), stop=False)
                # Bo-lora pair: tiles (0,0) and (32,0), concurrent
                nc.tensor.matmul(pfs[0][:], to[0:R + 1, nslc],
                                 bT["Bo"][0:R + 1, 0:F],
                                 start=False, stop=True)
                nc.tensor.matmul(pfs[1][:], to[32:33 + R, nslc],
                                 bT["Bo"][32:33 + R, F:2 * F],
                                 start=False, stop=True)
                for dh in range(2):
                    ds = slice(dh * F, (dh + 1) * F)
                    osb = stage.tile([P, F], bf16, tag="osb")
                    nc.scalar.copy(osb[:], pfs[dh][:])
                    nc.sync.dma_start(out=out_e[nslc, ds], in_=osb[:])
    nc.compile()
    return nc


def _get_nc():
    if "nc" not in _CACHE:
        _CACHE["nc"] = _build()
    return _CACHE["nc"]


def _prep_shared(inputs):
    def tb(a):  # transpose + bf16, contiguous
        return np.ascontiguousarray(np.asarray(a, np.float32).T.astype(BF16))

    shared = {}
    for nm in ("Wq", "Wk", "Wv", "Wo", "Ao"):
        shared[nm + "T"] = tb(inputs[nm])
    boa = np.zeros((49, D), np.float32)
    boa[0:R] = np.asarray(inputs["Bo"], np.float32).T
    boa[R] = np.asarray(inputs["bo"], np.float32)
    boa[32:32 + R] = boa[0:R]
    boa[32 + R] = boa[R]
    shared["BoT"] = np.ascontiguousarray(boa.astype(BF16))
    a3 = np.zeros((D, 96), np.float32)
    b3 = np.zeros((112, D), np.float32)
    for j, nm in enumerate(("q", "k", "v")):
        a3[:, 32 * j:32 * j + R] = np.asarray(inputs["A" + nm], np.float32).T
        b3[32 * j:32 * j + R, :] = np.asarray(inputs["B" + nm], np.float32).T
    b3[96:96 + R] = b3[64:64 + R]
    shared["A3T"] = np.ascontiguousarray(a3.astype(BF16))
    bm = np.zeros((33, P), np.float32)
    bm[0, 0:HD] = 1.0
    bm[32, HD:P] = 1.0
    shared["BM"] = np.ascontiguousarray(bm.astype(BF16))
    shared["B3T"] = np.ascontiguousarray(b3.astype(BF16))
    return shared


def kernel(**inputs):
    from concourse import bass_utils

    nc = _get_nc()
    shared = _prep_shared(inputs)
    x = np.asarray(inputs["x"], np.float32)
    in_maps = []
    for i in range(NCORES):
        m = dict(shared)
        m["xT"] = np.ascontiguousarray(x[i].T.astype(BF16))
        in_maps.append(m)
    res = bass_utils.run_bass_kernel_spmd(nc, in_maps,
                                          core_ids=list(range(NCORES)))
    return np.stack([np.asarray(res.results[i]["out"]).astype(np.float32)
                     for i in range(NCORES)], axis=0)
